# revision 1
# baseline (speedup 1.0000x reference)
"""GQA attention kernel for Trainium2, data-parallel over batch on 8 NeuronCores.

Per-core problem (2 of 16 batches): X [1024tok, 1024] -> QKV proj -> RoPE ->
causal GQA attention (8 q heads, 4 kv heads, D=128) -> out proj [1024, 1024].

Layout strategy (PE-occupancy-driven; ~147us, 1.4x over the phase-serial v1):
  - Whole-tensor input DMAs on the two HWDGE rings, ordered by first use
    (X on sync, Wk first on scalar) so the PE warmup covers exactly the
    runtime prologue (~7us) + X/Wk load (~9us).
  - Phase order: warmup -> K proj -> V proj -> [Q proj interleaved with
    attention, one attention (b,h) pair + two Q chains per head slot] ->
    [attention tail interleaved with out-projection chunks].  The PE never
    idles >3.4us, so the HAM clock gate stays at 8/8 (2.4 GHz) all kernel.
  - Attention is computed transposed (ST[tk,tq]): exp(ST) is directly the
    P.T operand of the PV matmul; denominator via a ones-column matmul
    chained in PSUM.  Causal masking: each PT_j plane stores columns [lo:T]
    shifted to plane-local [0:T-lo], so the four diagonal blocks of a head
    line up at [0:P] and one strided DVE multiply masks all of them.
  - Softmax normalization is per-head and fully off the PE: colsum psum ->
    DVE reciprocal_approx_fast -> GPSIMD partition_broadcast -> one DVE
    multiply that fuses the OT psum->sbuf copy with the 1/denom scale.
  - Engine assignment is FIFO-queue-driven (strict in-order queues: a slow
    dependency at the head blocks everything behind it):
      ACT:    exps, all psum->sbuf copies (qraw/V/out tiles)
      DVE:    merged causal masks, reciprocal, norm-multiply, rope mul/add
              (rope DVE ops emitted one head-slot late so their swap-DMA
              dependency is resolved before they reach the queue head)
      GPSIMD: partition_broadcast ONLY (any second op type forces ~5us Q7
              library reloads per switch)
      sync DMA ring: X load, rope half-swaps, output stores
      scalar DMA ring: all weight/const loads
  - A software pipeline (deque, drain threshold 11) keeps S/exp emission a
    full head ahead of the colsum/PV consumers, and attention consumption
    lags Q-chain emission by 3 heads, so exp+mask latency and the rope
    chain latency are never on the PE critical path.
"""

import numpy as np
import ml_dtypes
from collections import deque
from contextlib import ExitStack

import concourse.bass as bass
import concourse.tile as tile
from concourse import bacc, mybir
from concourse.bass_utils import run_bass_kernel_spmd

B, T, HID = 16, 512, 1024
NH, NKV, D = 8, 4, 128
THETA = 10000.0
NCORES = 8
BL = B // NCORES          # local batches per core
TOK = BL * T              # local tokens
P = 128
KT_HID = HID // P         # 8 contraction tiles over hidden
NTQ = T // P              # 4 tk/tq tiles per sequence
GROUPS = NH // NKV        # 2 q heads per kv head
FP32 = mybir.dt.float32
BF16 = mybir.dt.bfloat16
BF = ml_dtypes.bfloat16


def _host_consts():
    inv_freq = 1.0 / (THETA ** (np.arange(0, D, 2, dtype=np.float64) / D))
    freqs = np.outer(np.arange(T, dtype=np.float64), inv_freq)    # [T, 64]
    emb = np.concatenate([freqs, freqs], axis=-1)                 # [T, 128]
    cos = np.cos(emb).T                                           # [128, T]
    sin = np.sin(emb).T
    scale = 1.0 / np.sqrt(D)
    # rotate_half sign folded into sin: out = q*cos + qswap*sin_signed where
    # qswap is q with its partition halves swapped
    sin_signed = np.concatenate([-sin[:D // 2], sin[D // 2:]], axis=0)
    # both local batches are full T-length sequences -> tile along tokens
    cos2 = np.tile(cos, (1, BL))          # [128, TOK]
    sin2 = np.tile(sin_signed, (1, BL))
    # transposed-S diagonal-block multiplicative mask: rows tk, cols tq;
    # valid iff tq >= tk
    mask_t = np.triu(np.ones((P, P), np.float32)).astype(BF)
    return {
        "cos_q": (cos2 * scale).astype(BF),
        "sin_q": (sin2 * scale).astype(BF),
        "cos_k": cos2.astype(BF),
        "sin_k": sin2.astype(BF),
        "mask_t": mask_t,
    }


def _build(nc):
    # hidden arrives pre-transposed from the host: [HID, TOK]
    hid_t = nc.dram_tensor("hidden_t", [HID, TOK], BF16,
                           kind="ExternalInput").ap()
    wq = nc.dram_tensor("Wq", [HID, NH * D], BF16, kind="ExternalInput").ap()
    wk = nc.dram_tensor("Wk", [HID, NKV * D], BF16, kind="ExternalInput").ap()
    wv = nc.dram_tensor("Wv", [HID, NKV * D], BF16, kind="ExternalInput").ap()
    wo = nc.dram_tensor("Wo", [NH * D, HID], BF16, kind="ExternalInput").ap()
    cos_q = nc.dram_tensor("cos_q", [P, TOK], BF16, kind="ExternalInput").ap()
    sin_q = nc.dram_tensor("sin_q", [P, TOK], BF16, kind="ExternalInput").ap()
    cos_k = nc.dram_tensor("cos_k", [P, TOK], BF16, kind="ExternalInput").ap()
    sin_k = nc.dram_tensor("sin_k", [P, TOK], BF16, kind="ExternalInput").ap()
    mask_t = nc.dram_tensor("mask_t", [P, P], BF16, kind="ExternalInput").ap()
    out = nc.dram_tensor("out", [TOK, HID], FP32, kind="ExternalOutput").ap()

    NTOK_T = TOK // P   # 8 token tiles per core
    HALF = D // 2

    with tile.TileContext(nc) as tc, ExitStack() as ctx:
        # ---- pools with cross-phase lifetimes ----
        consts = ctx.enter_context(tc.tile_pool(name="consts", bufs=1))

        cosq_sb = consts.tile([P, TOK], BF16, tag="cq")
        sinq_sb = consts.tile([P, TOK], BF16, tag="sq")
        cosk_sb = consts.tile([P, TOK], BF16, tag="ck")
        sink_sb = consts.tile([P, TOK], BF16, tag="sk")
        maskt_sb = consts.tile([P, P], BF16, tag="maskt")
        ones_bf = consts.tile([P, P], BF16, tag="ones")
        nc.vector.memset(ones_bf, 1.0)
        warm_rhs = consts.tile([P, T], BF16, tag="warm")
        nc.vector.memset(warm_rhs, 0.0)

        qkvpool = ctx.enter_context(tc.tile_pool(name="qkv", bufs=1))
        qt_sb = qkvpool.tile([P, NH, TOK], BF16, tag="qt")     # [d, h, tok]
        kt_sb = qkvpool.tile([P, NKV, TOK], BF16, tag="kt")    # [d, g, tok]
        v_sb = qkvpool.tile([P, NTOK_T, NKV * D], BF16, tag="v")  # [tok,tt,dkv]
        otpool = ctx.enter_context(tc.tile_pool(name="otpool", bufs=1))
        ot_sb = otpool.tile([P, NH, TOK], BF16, tag="ot")      # [d, h, tok]

        wpool = ctx.enter_context(tc.tile_pool(name="wpool", bufs=1))
        wq_sb = wpool.tile([P, KT_HID, NH * D], BF16, tag="wq")
        wk_sb = wpool.tile([P, KT_HID, NKV * D], BF16, tag="wk")
        wv_sb = wpool.tile([P, KT_HID, NKV * D], BF16, tag="wv")
        wo_sb = wpool.tile([P, KT_HID, HID], BF16, tag="wo")
        xt_sb = wpool.tile([P, KT_HID, TOK], BF16, tag="xt")   # [hid, k, tok]

        # ---- input loads: whole-tensor DMAs, ordered by first use ----
        # sync ring: X only (so later rope-swap DMAs aren't queued behind
        # weight transfers); scalar ring: everything else in use order.
        hid_r = hid_t.rearrange("(k p) t -> p k t", p=P)
        wq_r = wq.rearrange("(k p) n -> p k n", p=P)
        wk_r = wk.rearrange("(k p) n -> p k n", p=P)
        wv_r = wv.rearrange("(k p) n -> p k n", p=P)
        wo_r = wo.rearrange("(k p) n -> p k n", p=P)
        nc.sync.dma_start(out=xt_sb[:, 0:2, :], in_=hid_r[:, 0:2, :])
        nc.sync.dma_start(out=xt_sb[:, 2:4, :], in_=hid_r[:, 2:4, :])
        nc.sync.dma_start(out=xt_sb[:, 4:6, :], in_=hid_r[:, 4:6, :])
        nc.scalar.dma_start(out=wk_sb[:, 0:4, :], in_=wk_r[:, 0:4, :])
        nc.scalar.dma_start(out=wk_sb[:, 4:8, :], in_=wk_r[:, 4:8, :])
        nc.scalar.dma_start(out=xt_sb[:, 6:8, :], in_=hid_r[:, 6:8, :])
        nc.scalar.dma_start(out=cosk_sb, in_=cos_k)
        nc.scalar.dma_start(out=sink_sb, in_=sin_k)
        nc.scalar.dma_start(out=wv_sb, in_=wv_r)
        nc.scalar.dma_start(out=wq_sb[:, 0:4, :], in_=wq_r[:, 0:4, :])
        nc.scalar.dma_start(out=wq_sb[:, 4:8, :], in_=wq_r[:, 4:8, :])
        nc.scalar.dma_start(out=cosq_sb, in_=cos_q)
        nc.scalar.dma_start(out=sinq_sb, in_=sin_q)
        nc.scalar.dma_start(out=maskt_sb, in_=mask_t)
        nc.scalar.dma_start(out=wo_sb, in_=wo_r)

        def _rope_swap_part(raw, sin_sb, tmp_pool):
            """Launch the latency-y half of rope: partition-half swap on the
            sync DMA ring (compute engines cannot shift partitions).
            Returns the swap tile for _rope_dve_part."""
            qswap = tmp_pool.tile([P, TOK], BF16, tag="rope_swap", bufs=2,
                                  name="qswap")
            nc.sync.dma_start(out=qswap[0:HALF], in_=raw[HALF:P])
            nc.sync.dma_start(out=qswap[HALF:P], in_=raw[0:HALF])
            return qswap

        def _rope_dve_part(raw, qswap, out_sl, cos_sb, sin_sb, tmp_pool):
            """DVE tail of rope: out = raw * cos + qswap * sin.  Emitted a
            full head slot after _rope_swap_part so its dependencies are
            long resolved and it never idles the DVE queue ahead of the
            causal masks."""
            tmp = tmp_pool.tile([P, TOK], BF16, tag="rope_tmp", bufs=2,
                                name="tmp")
            nc.vector.tensor_mul(tmp, qswap, sin_sb)
            nc.vector.tensor_mul(out_sl, raw, cos_sb)
            nc.vector.tensor_add(out_sl, out_sl, tmp)

        # ---- phase A: warmup + K proj + V proj ----
        with ExitStack() as phase1:
            ropet = phase1.enter_context(tc.tile_pool(name="ropetA", bufs=2))
            psA = phase1.enter_context(
                tc.tile_pool(name="psA", bufs=6, space=bass.MemorySpace.PSUM))
            psW = phase1.enter_context(
                tc.tile_pool(name="psW", bufs=1, space=bass.MemorySpace.PSUM))

            # PE warmup: ~9us of dependency-free matmuls so the HAM clock
            # gate releases (1.2 -> 2.4 GHz) while X+Wk are still in flight
            wps = psW.tile([P, T], FP32, tag="warmps")
            for w in range(30):
                nc.tensor.matmul(wps, ones_bf, warm_rhs,
                                 start=True, stop=True, skip_group_check=True)
            for w in range(8):
                nc.tensor.matmul(wps[:, 0:P], ones_bf, warm_rhs[:, 0:P],
                                 start=True, stop=True, skip_group_check=True)

            # KT = Wk.T @ XT + RoPE (rope merged over both batch chunks)
            for g in range(NKV):
                kraw = ropet.tile([P, TOK], BF16, tag="rope_raw", bufs=2,
                                  name="kraw")
                for c in range(BL):
                    ps = psA.tile([P, T], FP32, tag="projps")
                    for k in range(KT_HID):
                        nc.tensor.matmul(
                            ps,
                            wk_sb[:, k, g * P:(g + 1) * P],
                            xt_sb[:, k, c * T:(c + 1) * T],
                            start=(k == 0), stop=(k == KT_HID - 1))
                    nc.scalar.copy(kraw[:, c * T:(c + 1) * T], ps)
                kswap = _rope_swap_part(kraw, sink_sb, ropet)
                _rope_dve_part(kraw, kswap, kt_sb[:, g, :], cosk_sb,
                               sink_sb, ropet)
            # V natural: [tok, dkv]
            for tt in range(NTOK_T):
                ps = psA.tile([P, T], FP32, tag="projps")
                for k in range(KT_HID):
                    nc.tensor.matmul(
                        ps[:, :NKV * D],
                        xt_sb[:, k, tt * P:(tt + 1) * P],
                        wv_sb[:, k, :],
                        start=(k == 0), stop=(k == KT_HID - 1))
                nc.scalar.copy(v_sb[:, tt, :], ps[:, :NKV * D])

        # ---- phase B: Q proj interleaved with attention ----
        with ExitStack() as phase2:
            ropet = phase2.enter_context(tc.tile_pool(name="ropetB", bufs=2))
            ptpool = phase2.enter_context(tc.tile_pool(name="ptpool", bufs=5))
            stats = phase2.enter_context(tc.tile_pool(name="stats", bufs=3))
            psM = phase2.enter_context(
                tc.tile_pool(name="psM", bufs=1, space=bass.MemorySpace.PSUM))

            pend = deque()
            head_state = {}
            qraw_state = {}
            rope_pending = deque()

            def emit_qchain(h, c):
                ps = psM.tile([P, T], FP32, tag="qps", bufs=2, name="qps")
                for k in range(KT_HID):
                    nc.tensor.matmul(
                        ps,
                        wq_sb[:, k, h * P:(h + 1) * P],
                        xt_sb[:, k, c * T:(c + 1) * T],
                        start=(k == 0), stop=(k == KT_HID - 1))
                if c == 0:
                    qraw_state[h] = ropet.tile([P, TOK], BF16, tag="rope_raw",
                                               bufs=2, name="qraw")
                qraw = qraw_state[h]
                nc.scalar.copy(qraw[:, c * T:(c + 1) * T], ps)
                if c == BL - 1:
                    qswap = _rope_swap_part(qraw, sinq_sb, ropet)
                    rope_pending.append((qraw, qswap, h))
                    del qraw_state[h]

            def emit_rope_dve():
                qraw, qswap, h = rope_pending.popleft()
                _rope_dve_part(qraw, qswap, qt_sb[:, h, :], cosq_sb,
                               sinq_sb, ropet)

            def emit_item_S(b, h):
                """S matmuls + exps for all 4 tk-blocks of one (batch, head),
                then ONE merged causal-mask multiply: each PT_j plane stores
                columns [lo:T] shifted to plane-local [0:T-lo], so all four
                diagonal blocks line up at plane-local [0:P] and mask in a
                single strided DVE op (4x fewer mask ops, and the drains lag
                far enough that mask latency never stalls the PE)."""
                g = h // GROUPS
                pt = ptpool.tile([P, NTQ, T], BF16, tag="pt", bufs=5,
                                 name="pt")
                for j in range(NTQ):
                    lo = j * P
                    st_ps = psM.tile([P, T], FP32, tag="sps", bufs=3,
                                     name="sps")
                    nc.tensor.matmul(
                        st_ps[:, lo:T],
                        kt_sb[:, g, b * T + lo: b * T + lo + P],
                        qt_sb[:, h, b * T + lo: (b + 1) * T],
                        start=True, stop=True)
                    # exp -> PT_j, already transposed for the PV matmul
                    # (no row-max: logits are O(1) by construction)
                    nc.scalar.activation(
                        out=pt[:, j, 0:T - lo], in_=st_ps[:, lo:T],
                        func=mybir.ActivationFunctionType.Exp,
                        bias=0.0, scale=1.0)
                nc.vector.tensor_mul(
                    pt[:, :, 0:P], pt[:, :, 0:P],
                    maskt_sb[:, None, :].to_broadcast([P, NTQ, P]))
                for j in range(NTQ):
                    pend.append((b, h, j, pt))

            def drain_one():
                b, h, j, pt = pend.popleft()
                g = h // GROUPS
                lo = j * P
                st = head_state.get((b, h))
                if st is None:
                    o_ps_new = psM.tile([P, T], FP32, tag="ops", bufs=2,
                                        name="ops")
                    cs_ps_new = psM.tile([1, T], FP32, tag="cps", bufs=1,
                                         name="cps")
                    st = head_state[(b, h)] = (o_ps_new, cs_ps_new)
                o_ps, cs_ps = st
                # colsum += ones.T @ PT_j ; OT += V_j.T @ PT_j
                nc.tensor.matmul(
                    cs_ps[:, lo:T] if j else cs_ps[:, :],
                    ones_bf[:, 0:1],
                    pt[:, j, 0:T - lo],
                    start=(j == 0), stop=(j == NTQ - 1),
                    skip_group_check=True)
                nc.tensor.matmul(
                    o_ps[:, lo:T] if j else o_ps[:, :],
                    v_sb[:, b * NTQ + j, g * D:(g + 1) * D],
                    pt[:, j, 0:T - lo],
                    start=(j == 0), stop=(j == NTQ - 1),
                    skip_group_check=True)
                if j == NTQ - 1:
                    # per-head softmax normalization, entirely off the PE:
                    # 1/colsum -> broadcast over partitions -> fused into the
                    # OT psum->sbuf copy
                    rr = stats.tile([1, T], FP32, tag="rr")
                    nc.vector.reciprocal_approx_fast(rr, cs_ps)
                    # partition-broadcast on GPSIMD: with all rope
                    # arithmetic moved to the DVE this is GPSIMD's only op
                    # type, so no Q7 library reloads occur
                    rb = stats.tile([P, T], FP32, tag="rb")
                    nc.gpsimd.partition_broadcast(rb, rr)
                    nc.vector.tensor_mul(
                        ot_sb[:, h, b * T:(b + 1) * T], o_ps, rb)
                    del head_state[(b, h)]

            def emit_att(b, h):
                emit_item_S(b, h)
                while len(pend) > 11:
                    drain_one()

            def emit_oproj(tt, cchunk):
                # one out-projection chunk: out[tt-block, chunk] as its own
                # k-chain, sharing the qps psum rotation with the (finished)
                # Q chains so it can interleave with the attention tail
                ps = psM.tile([P, T], FP32, tag="qps", bufs=2, name="opps")
                for k in range(KT_HID):
                    nc.tensor.matmul(
                        ps,
                        ot_sb[:, k, tt * P:(tt + 1) * P],
                        wo_sb[:, k, cchunk * T:(cchunk + 1) * T],
                        start=(k == 0), stop=(k == KT_HID - 1))
                o_tile = stats.tile([P, T], FP32, tag="oout", bufs=3,
                                    name="o_tile")
                if tt == NTOK_T - 1:
                    # final tile: split the drain across both copy engines
                    # and both DMA rings to shorten the kernel epilogue
                    HT = T // 2
                    nc.scalar.copy(o_tile[:, 0:HT], ps[:, 0:HT])
                    nc.vector.tensor_copy(o_tile[:, HT:T], ps[:, HT:T])
                    base = cchunk * T
                    nc.sync.dma_start(
                        out=out[tt * P:(tt + 1) * P, base:base + HT],
                        in_=o_tile[:, 0:HT])
                    nc.scalar.dma_start(
                        out=out[tt * P:(tt + 1) * P, base + HT:base + T],
                        in_=o_tile[:, HT:T])
                    return
                # copies always on ACT: a PE-dependent copy on the DVE would
                # head-of-line delay the norm-mults that gate later chains
                nc.scalar.copy(o_tile, ps)
                eng = nc.sync if (2 * tt + cchunk) % 2 == 0 else nc.scalar
                eng.dma_start(
                    out=out[tt * P:(tt + 1) * P,
                            cchunk * T:(cchunk + 1) * T],
                    in_=o_tile)

            # attention lags the Q chains by 3 heads and the DVE part of
            # each rope lags its chain by 1 slot: every engine-queue entry
            # has its dependencies resolved before it reaches the queue
            # head, so the strict-FIFO DVE/GPSIMD queues never head-of-line
            # block the causal masks that gate the PE's colsum/PV matmuls
            LAG = 3
            for h in range(NH):
                if h >= LAG:
                    emit_att(0, h - LAG)
                emit_qchain(h, 0)
                if h >= LAG:
                    emit_att(1, h - LAG)
                if rope_pending and h >= 1:
                    emit_rope_dve()
                emit_qchain(h, 1)
            while rope_pending:
                emit_rope_dve()
            # tail: batch-0 attention first, then interleave out-projection
            # chunks (batch-0 token tiles first) with the remaining
            # ACT/DVE-bound attention so the PE stays dense and the HAM
            # clock gate never re-throttles
            for h in range(NH - LAG, NH):
                emit_att(0, h)
            emit_att(1, NH - LAG)
            while len(pend) > 4:
                drain_one()              # flush: norms (0, *) all emitted
            emit_oproj(0, 0)
            emit_oproj(0, 1)
            emit_att(1, NH - 2)
            emit_oproj(1, 0)
            emit_oproj(1, 1)
            emit_att(1, NH - 1)
            emit_oproj(2, 0)
            emit_oproj(2, 1)
            while len(pend) > 4:
                drain_one()              # flush: norm (1, NH-2) emitted
            emit_oproj(3, 0)
            while pend:
                drain_one()              # norm (1, NH-1)
            emit_oproj(3, 1)
            for tt in range(4, NTOK_T):
                emit_oproj(tt, 0)
                emit_oproj(tt, 1)


_COMPILED = None


def _get_compiled():
    global _COMPILED
    if _COMPILED is None:
        nc = bacc.Bacc("TRN2", target_bir_lowering=False, debug=False)
        _build(nc)
        nc.compile()
        _COMPILED = nc
    return _COMPILED


def kernel(hidden_states, Wq, Wk, Wv, Wo, _trace=False, _trace_kwargs=None):
    hs = np.asarray(hidden_states, dtype=np.float32).astype(BF)
    wq = np.ascontiguousarray(np.asarray(Wq, dtype=np.float32).astype(BF))
    wk = np.ascontiguousarray(np.asarray(Wk, dtype=np.float32).astype(BF))
    wv = np.ascontiguousarray(np.asarray(Wv, dtype=np.float32).astype(BF))
    wo = np.ascontiguousarray(np.asarray(Wo, dtype=np.float32).astype(BF))
    consts = _host_consts()
    nc = _get_compiled()
    in_maps = []
    for c in range(NCORES):
        # ship X pre-transposed ([HID, TOK]) so the kernel's lhs/rhs layouts
        # need no on-chip transpose of X at all
        shard_t = np.ascontiguousarray(
            hs[BL * c: BL * (c + 1)].reshape(TOK, HID).T)
        in_maps.append({"hidden_t": shard_t, "Wq": wq, "Wk": wk, "Wv": wv,
                        "Wo": wo, **consts})
    res = run_bass_kernel_spmd(
        nc, in_maps, list(range(NCORES)), trace=_trace,
        **(_trace_kwargs or {}))
    outs = [r["out"].astype(np.float32).reshape(BL, T, HID)
            for r in res.results]
    full = np.concatenate(outs, axis=0)
    if _trace:
        return full, res
    return full



# revision 4
# speedup vs baseline: 1.0727x; 1.0727x over previous
"""GQA attention kernel for Trainium2, data-parallel over batch on 8 NeuronCores.

Per-core problem (2 of 16 batches): X [1024tok, 1024] -> QKV proj -> RoPE ->
causal GQA attention (8 q heads, 4 kv heads, D=128) -> out proj [1024, 1024].

v3 layout strategy (PE-occupancy-driven; baseline v2 was ~146.4us):
  - All dram tensors are host-side pre-arranged to the exact sbuf fill
    layout ([p, k, n]) so every load DMA runs with 4-16KB contiguous
    descriptors, and loads are ordered by first use with the K-projection
    dependencies (X, then per-g Wk chunks) first across both HWDGE rings.
  - RoPE's partition-half swap is done ON THE DVE via stream_shuffle: the
    head dim of Wq/Wk (and cos/sin rows) is permuted so each rotate-half
    pair (i, i+64) lands 16 partitions apart inside one 32-partition
    quadrant (S = q.k is invariant under a consistent d-permutation).
    This removes all sbuf<->sbuf swap DMAs (3MB of ring traffic that used
    to compete with the weight loads) and makes rope a pure DVE chain.
  - The 1/sqrt(D) scale is folded into the exp's activation scale, so one
    UNSCALED cos/sin table pair [128, 512] is shared by Q and K rope and
    broadcast over the two batch chunks (0.25MB loaded vs 1MB).
  - Softmax denominator: the colsum matmul uses an ALL-ONES [128,128]
    stationary operand, so the psum result is the denominator already
    broadcast across partitions (same PE streaming cost, cheaper
    instruction shape than M=1) and normalization is a DVE
    reciprocal_approx_fast + one multiply -- no GPSIMD
    partition_broadcast; GPSIMD retires from the kernel entirely.
    (A single tensor_tensor divide would be cheaper still but the BIR
    verifier rejects divide on the DVE.)
  - PE warmup is dependency-light (ones@ones after a gpsimd memset) so it
    starts as soon as the PE queue comes up (~6.6us) and is sized to end
    exactly when X+Wk land (~15us), covering the whole load latency.
  - Output dram tensor is bf16 (host upcasts): halves store traffic, and
    the last two out-projection chunks drain in [128,128] quarters
    alternating ACT/DVE copies and sync/scalar rings to shorten the tail.
  - Engine assignment is FIFO-queue-driven (strict in-order queues):
      ACT:    exps, psum->sbuf copies (qraw/V/out tiles)
      DVE:    rope (shuffle+mul+mul+add), merged causal masks, divides
      GPSIMD: nothing (only the startup ones memset)
      sync/scalar DMA rings: loads first-use-ordered, then output stores
  - Same software pipeline as v2: S/exp emission runs a full head ahead of
    the colsum/PV consumers (deque, drain threshold 11), attention
    consumption lags Q-chain emission by 3 heads, rope DVE chains are
    emitted one head-slot late.
"""

import numpy as np
import ml_dtypes
from collections import deque
from contextlib import ExitStack

import concourse.bass as bass
import concourse.tile as tile
from concourse import bacc, mybir
from concourse.bass_utils import run_bass_kernel_spmd

B, T, HID = 16, 512, 1024
NH, NKV, D = 8, 4, 128
THETA = 10000.0
NCORES = 8
BL = B // NCORES          # local batches per core
TOK = BL * T              # local tokens
P = 128
KT_HID = HID // P         # 8 contraction tiles over hidden
NTQ = T // P              # 4 tk/tq tiles per sequence
NTOK_T = TOK // P         # 8 token tiles per core
GROUPS = NH // NKV        # 2 q heads per kv head
SCALE = 1.0 / float(np.sqrt(D))
FP32 = mybir.dt.float32
BF16 = mybir.dt.bfloat16
BF = ml_dtypes.bfloat16

# rope-pair permutation: old pair (i, i+64) -> within-quadrant pair
# (32q+j, 32q+16+j) with q=i//16, j=i%16, so one stream_shuffle mask
# (swap 16-partition halves of each 32-partition quadrant) does the
# rotate-half partition move on the DVE.
_DPERM = np.empty(D, dtype=np.int64)          # old index of each new slot
for _q in range(4):
    for _j in range(16):
        _DPERM[32 * _q + _j] = 16 * _q + _j
        _DPERM[32 * _q + 16 + _j] = 64 + 16 * _q + _j
SHUF_MASK = list(range(16, 32)) + list(range(0, 16))

WARM = 72                 # PE warmup matmuls (ones[P,P] @ ones[P,P])


def _host_consts():
    inv_freq = 1.0 / (THETA ** (np.arange(0, D, 2, dtype=np.float64) / D))
    freqs = np.outer(np.arange(T, dtype=np.float64), inv_freq)    # [T, 64]
    emb = np.concatenate([freqs, freqs], axis=-1)                 # [T, 128]
    cos = np.cos(emb).T                                           # [128, T]
    sin = np.sin(emb).T
    # rotate_half sign folded into sin: out = x*cos + shuffle(x)*sin_signed
    sin_signed = np.concatenate([-sin[:D // 2], sin[D // 2:]], axis=0)
    # transposed-S diagonal-block multiplicative mask: rows tk, cols tq;
    # valid iff tq >= tk
    mask_t = np.triu(np.ones((P, P), np.float32)).astype(BF)
    return {
        "cos_t": np.ascontiguousarray(cos[_DPERM]).astype(BF),
        "sin_t": np.ascontiguousarray(sin_signed[_DPERM]).astype(BF),
        "mask_t": mask_t,
    }


def _build(nc):
    hid = nc.dram_tensor("hidden_pk", [P, KT_HID, TOK], BF16,
                         kind="ExternalInput").ap()
    wq = nc.dram_tensor("wq_pk", [P, KT_HID, NH * D], BF16,
                        kind="ExternalInput").ap()
    wk = nc.dram_tensor("wk_g", [NKV, P, KT_HID, D], BF16,
                        kind="ExternalInput").ap()
    wv = nc.dram_tensor("wv_pk", [P, KT_HID, NKV * D], BF16,
                        kind="ExternalInput").ap()
    wo = nc.dram_tensor("wo_pk", [P, KT_HID, HID], BF16,
                        kind="ExternalInput").ap()
    cos_t = nc.dram_tensor("cos_t", [P, T], BF16, kind="ExternalInput").ap()
    sin_t = nc.dram_tensor("sin_t", [P, T], BF16, kind="ExternalInput").ap()
    mask_t = nc.dram_tensor("mask_t", [P, P], BF16, kind="ExternalInput").ap()
    out = nc.dram_tensor("out", [TOK, HID], BF16, kind="ExternalOutput").ap()

    with tile.TileContext(nc) as tc, ExitStack() as ctx:
        # ---- pools with cross-phase lifetimes ----
        consts = ctx.enter_context(tc.tile_pool(name="consts", bufs=1))

        cos_sb = consts.tile([P, T], BF16, tag="cos")
        sin_sb = consts.tile([P, T], BF16, tag="sin")
        maskt_sb = consts.tile([P, P], BF16, tag="maskt")
        ones_bf = consts.tile([P, P], BF16, tag="ones")
        # gpsimd comes up first (~6.1us) -> warmup deps ready earliest
        nc.gpsimd.memset(ones_bf, 1.0)

        qkvpool = ctx.enter_context(tc.tile_pool(name="qkv", bufs=1))
        qt_sb = qkvpool.tile([P, NH, BL, T], BF16, tag="qt")    # [d,h,b,t]
        kt_sb = qkvpool.tile([P, NKV, BL, T], BF16, tag="kt")   # [d,g,b,t]
        v_sb = qkvpool.tile([P, NTOK_T, NKV * D], BF16, tag="v")
        otpool = ctx.enter_context(tc.tile_pool(name="otpool", bufs=1))
        ot_sb = otpool.tile([P, NH, BL, T], BF16, tag="ot")     # [d,h,b,t]

        wpool = ctx.enter_context(tc.tile_pool(name="wpool", bufs=1))
        wq_sb = wpool.tile([P, KT_HID, NH * D], BF16, tag="wq")
        wk_sb = wpool.tile([P, NKV, KT_HID, D], BF16, tag="wk")
        wv_sb = wpool.tile([P, KT_HID, NKV * D], BF16, tag="wv")
        wo_sb = wpool.tile([P, KT_HID, HID], BF16, tag="wo")
        xt_sb = wpool.tile([P, KT_HID, TOK], BF16, tag="xt")    # [hid,k,tok]

        # ---- input loads: first-use order, K-proj deps (X, per-g Wk)
        # first; everything is contiguous in dram per partition ----
        nc.sync.dma_start(out=xt_sb[:, 0:2, :], in_=hid[:, 0:2, :])
        nc.scalar.dma_start(out=xt_sb[:, 4:6, :], in_=hid[:, 4:6, :])
        nc.sync.dma_start(out=xt_sb[:, 2:4, :], in_=hid[:, 2:4, :])
        nc.scalar.dma_start(out=xt_sb[:, 6:8, :], in_=hid[:, 6:8, :])
        nc.sync.dma_start(out=wk_sb[:, 0], in_=wk[0])
        nc.scalar.dma_start(out=wk_sb[:, 1], in_=wk[1])
        nc.sync.dma_start(out=wk_sb[:, 2], in_=wk[2])
        nc.scalar.dma_start(out=wk_sb[:, 3], in_=wk[3])
        nc.sync.dma_start(out=cos_sb, in_=cos_t)
        nc.sync.dma_start(out=sin_sb, in_=sin_t)
        nc.scalar.dma_start(out=wv_sb, in_=wv)
        nc.sync.dma_start(out=wq_sb[:, 0:4, :], in_=wq[:, 0:4, :])
        nc.scalar.dma_start(out=wq_sb[:, 4:8, :], in_=wq[:, 4:8, :])
        nc.sync.dma_start(out=maskt_sb, in_=mask_t)
        nc.sync.dma_start(out=wo_sb[:, 0:4, :], in_=wo[:, 0:4, :])
        nc.scalar.dma_start(out=wo_sb[:, 4:8, :], in_=wo[:, 4:8, :])

        cos_bc = cos_sb[:, None, :].to_broadcast([P, BL, T])
        sin_bc = sin_sb[:, None, :].to_broadcast([P, BL, T])

        def _rope_dve(raw, out_sl, tmp_pool):
            """Full rope on the DVE: partition-half swap via stream_shuffle
            (head-dim permuted so pairs sit within 32-partition quadrants),
            then out = raw*cos + shuffled*sin_signed."""
            swp = tmp_pool.tile([P, BL, T], BF16, tag="rope_swp", bufs=2,
                                name="swp")
            nc.vector.stream_shuffle(swp, raw, SHUF_MASK)
            tmp = tmp_pool.tile([P, BL, T], BF16, tag="rope_tmp", bufs=2,
                                name="tmp")
            nc.vector.tensor_mul(tmp, swp, sin_bc)
            nc.vector.tensor_mul(out_sl, raw, cos_bc)
            nc.vector.tensor_add(out_sl, out_sl, tmp)

        # ---- phase A: warmup + K proj + V proj ----
        with ExitStack() as phase1:
            ropet = phase1.enter_context(tc.tile_pool(name="ropetA", bufs=2))
            psA = phase1.enter_context(
                tc.tile_pool(name="psA", bufs=6, space=bass.MemorySpace.PSUM))
            psW = phase1.enter_context(
                tc.tile_pool(name="psW", bufs=1, space=bass.MemorySpace.PSUM))

            # PE warmup: dependency-free matmuls so the PE queue ramps the
            # HAM clock gate (1.2 -> 2.4 GHz) while X+Wk are in flight
            wps = psW.tile([P, P], FP32, tag="warmps")
            for w in range(WARM):
                nc.tensor.matmul(wps, ones_bf, ones_bf,
                                 start=True, stop=True, skip_group_check=True)

            # KT = Wk.T @ XT + RoPE (both batch chunks merged per g)
            for g in range(NKV):
                kraw = ropet.tile([P, BL, T], BF16, tag="rope_raw", bufs=2,
                                  name="kraw")
                for c in range(BL):
                    ps = psA.tile([P, T], FP32, tag="projps")
                    for k in range(KT_HID):
                        nc.tensor.matmul(
                            ps,
                            wk_sb[:, g, k, :],
                            xt_sb[:, k, c * T:(c + 1) * T],
                            start=(k == 0), stop=(k == KT_HID - 1))
                    nc.scalar.copy(kraw[:, c, :], ps)
                _rope_dve(kraw, kt_sb[:, g], ropet)
            # V natural: [tok, dkv]
            for tt in range(NTOK_T):
                ps = psA.tile([P, T], FP32, tag="projps")
                for k in range(KT_HID):
                    nc.tensor.matmul(
                        ps[:, :NKV * D],
                        xt_sb[:, k, tt * P:(tt + 1) * P],
                        wv_sb[:, k, :],
                        start=(k == 0), stop=(k == KT_HID - 1))
                nc.scalar.copy(v_sb[:, tt, :], ps[:, :NKV * D])

        # ---- phase B: Q proj interleaved with attention ----
        with ExitStack() as phase2:
            ropet = phase2.enter_context(tc.tile_pool(name="ropetB", bufs=2))
            ptpool = phase2.enter_context(tc.tile_pool(name="ptpool", bufs=5))
            stats = phase2.enter_context(tc.tile_pool(name="stats", bufs=3))
            psM = phase2.enter_context(
                tc.tile_pool(name="psM", bufs=1, space=bass.MemorySpace.PSUM))

            pend = deque()
            head_state = {}
            qraw_state = {}
            rope_pending = deque()

            def emit_qchain(h, c):
                ps = psM.tile([P, T], FP32, tag="qps", bufs=2, name="qps")
                for k in range(KT_HID):
                    nc.tensor.matmul(
                        ps,
                        wq_sb[:, k, h * P:(h + 1) * P],
                        xt_sb[:, k, c * T:(c + 1) * T],
                        start=(k == 0), stop=(k == KT_HID - 1))
                if c == 0:
                    qraw_state[h] = ropet.tile([P, BL, T], BF16,
                                               tag="rope_raw", bufs=2,
                                               name="qraw")
                qraw = qraw_state[h]
                nc.scalar.copy(qraw[:, c, :], ps)
                if c == BL - 1:
                    rope_pending.append((qraw, h))
                    del qraw_state[h]

            def emit_rope_dve():
                qraw, h = rope_pending.popleft()
                _rope_dve(qraw, qt_sb[:, h], ropet)

            def emit_item_S(b, h):
                """S matmuls + exps for all 4 tk-blocks of one (batch, head),
                then ONE merged causal-mask multiply: each PT_j plane stores
                columns [lo:T] shifted to plane-local [0:T-lo], so all four
                diagonal blocks line up at plane-local [0:P] and mask in a
                single strided DVE op."""
                g = h // GROUPS
                pt = ptpool.tile([P, NTQ, T], BF16, tag="pt", bufs=5,
                                 name="pt")
                for j in range(NTQ):
                    lo = j * P
                    st_ps = psM.tile([P, T], FP32, tag="sps", bufs=3,
                                     name="sps")
                    nc.tensor.matmul(
                        st_ps[:, lo:T],
                        kt_sb[:, g, b, lo:lo + P],
                        qt_sb[:, h, b, lo:T],
                        start=True, stop=True)
                    # exp -> PT_j, already transposed for the PV matmul;
                    # 1/sqrt(D) folded into the activation scale
                    # (no row-max: logits are O(1) by construction)
                    nc.scalar.activation(
                        out=pt[:, j, 0:T - lo], in_=st_ps[:, lo:T],
                        func=mybir.ActivationFunctionType.Exp,
                        bias=0.0, scale=SCALE)
                nc.vector.tensor_mul(
                    pt[:, :, 0:P], pt[:, :, 0:P],
                    maskt_sb[:, None, :].to_broadcast([P, NTQ, P]))
                for j in range(NTQ):
                    pend.append((b, h, j, pt))

            def drain_one():
                b, h, j, pt = pend.popleft()
                g = h // GROUPS
                lo = j * P
                st = head_state.get((b, h))
                if st is None:
                    o_ps_new = psM.tile([P, T], FP32, tag="ops", bufs=2,
                                        name="ops")
                    cs_ps_new = psM.tile([P, T], FP32, tag="cps", bufs=1,
                                         name="cps")
                    st = head_state[(b, h)] = (o_ps_new, cs_ps_new)
                o_ps, cs_ps = st
                # colsum with an ALL-ONES stationary: psum rows all carry
                # the denominator (partition-broadcast for free);
                # OT += V_j.T @ PT_j chained in the other psum
                nc.tensor.matmul(
                    cs_ps[:, lo:T] if j else cs_ps[:, :],
                    ones_bf,
                    pt[:, j, 0:T - lo],
                    start=(j == 0), stop=(j == NTQ - 1),
                    skip_group_check=True)
                nc.tensor.matmul(
                    o_ps[:, lo:T] if j else o_ps[:, :],
                    v_sb[:, b * NTQ + j, g * D:(g + 1) * D],
                    pt[:, j, 0:T - lo],
                    start=(j == 0), stop=(j == NTQ - 1),
                    skip_group_check=True)
                if j == NTQ - 1:
                    # per-head softmax normalization, fully on the DVE: the
                    # all-ones colsum already broadcast the denominator
                    # across partitions, so reciprocal + one multiply
                    rb = stats.tile([P, T], FP32, tag="rb", bufs=2,
                                    name="rb")
                    nc.vector.reciprocal_approx_fast(rb, cs_ps)
                    nc.vector.tensor_mul(ot_sb[:, h, b], o_ps, rb)
                    del head_state[(b, h)]

            def emit_att(b, h):
                emit_item_S(b, h)
                while len(pend) > 11:
                    drain_one()

            def emit_oproj(tt, cchunk):
                # one out-projection chunk: out[tt-block, chunk] as its own
                # k-chain, sharing the qps psum rotation with the (finished)
                # Q chains so it can interleave with the attention tail
                ps = psM.tile([P, T], FP32, tag="qps", bufs=2, name="opps")
                for k in range(KT_HID):
                    nc.tensor.matmul(
                        ps,
                        ot_sb[:, k, tt // NTQ,
                              (tt % NTQ) * P:(tt % NTQ + 1) * P],
                        wo_sb[:, k, cchunk * T:(cchunk + 1) * T],
                        start=(k == 0), stop=(k == KT_HID - 1))
                o_tile = stats.tile([P, T], BF16, tag="oout", bufs=3,
                                    name="o_tile")
                if tt >= NTOK_T - 1:
                    # final tiles: drain in quarters alternating both copy
                    # engines and both DMA rings to shorten the epilogue
                    QT = T // 4
                    for q in range(4):
                        sl = slice(q * QT, (q + 1) * QT)
                        if q % 2 == 0:
                            nc.scalar.copy(o_tile[:, sl], ps[:, sl])
                        else:
                            nc.vector.tensor_copy(o_tile[:, sl], ps[:, sl])
                        eng = nc.sync if q % 2 == 0 else nc.scalar
                        base = cchunk * T
                        eng.dma_start(
                            out=out[tt * P:(tt + 1) * P,
                                    base + q * QT:base + (q + 1) * QT],
                            in_=o_tile[:, sl])
                    return
                # copies always on ACT: a PE-dependent copy on the DVE would
                # head-of-line delay the divides that gate later chains
                nc.scalar.copy(o_tile, ps)
                eng = nc.sync if (2 * tt + cchunk) % 2 == 0 else nc.scalar
                eng.dma_start(
                    out=out[tt * P:(tt + 1) * P,
                            cchunk * T:(cchunk + 1) * T],
                    in_=o_tile)

            # attention lags the Q chains by 3 heads and the DVE part of
            # each rope lags its chain by 1 slot: every engine-queue entry
            # has its dependencies resolved before it reaches the queue
            # head, so the strict-FIFO DVE queue never head-of-line blocks
            # the causal masks that gate the PE's colsum/PV matmuls
            LAG = 3
            for h in range(NH):
                if h >= LAG:
                    emit_att(0, h - LAG)
                emit_qchain(h, 0)
                if h >= LAG:
                    emit_att(1, h - LAG)
                if rope_pending and h >= 1:
                    emit_rope_dve()
                emit_qchain(h, 1)
            while rope_pending:
                emit_rope_dve()
            # tail: batch-0 attention first, then interleave out-projection
            # chunks (batch-0 token tiles first) with the remaining
            # ACT/DVE-bound attention so the PE stays dense
            for h in range(NH - LAG, NH):
                emit_att(0, h)
            emit_att(1, NH - LAG)
            while len(pend) > 4:
                drain_one()              # flush: norms (0, *) all emitted
            emit_oproj(0, 0)
            emit_oproj(0, 1)
            emit_att(1, NH - 2)
            emit_oproj(1, 0)
            emit_oproj(1, 1)
            emit_att(1, NH - 1)
            emit_oproj(2, 0)
            emit_oproj(2, 1)
            while len(pend) > 4:
                drain_one()              # flush: norm (1, NH-2) emitted
            emit_oproj(3, 0)
            while pend:
                drain_one()              # norm (1, NH-1)
            emit_oproj(3, 1)
            for tt in range(4, NTOK_T):
                emit_oproj(tt, 0)
                emit_oproj(tt, 1)


def _prep_weights(Wq, Wk, Wv, Wo):
    """Host-side: bf16, rope-pair permutation of the q/k head dims, and
    rearrangement to the sbuf fill layout [p, k, n] (contiguous per
    partition) so load DMA descriptors are large."""
    wq = np.asarray(Wq, dtype=np.float32).astype(BF)
    wk = np.asarray(Wk, dtype=np.float32).astype(BF)
    wv = np.asarray(Wv, dtype=np.float32).astype(BF)
    wo = np.asarray(Wo, dtype=np.float32).astype(BF)
    # permute the per-head d axis of Wq/Wk
    wq = wq.reshape(HID, NH, D)[:, :, _DPERM].reshape(HID, NH * D)
    wk = wk.reshape(HID, NKV, D)[:, :, _DPERM].reshape(HID, NKV * D)
    wq_pk = np.ascontiguousarray(
        wq.reshape(KT_HID, P, NH * D).transpose(1, 0, 2))
    wk_g = np.ascontiguousarray(
        wk.reshape(KT_HID, P, NKV, D).transpose(2, 1, 0, 3))
    wv_pk = np.ascontiguousarray(
        wv.reshape(KT_HID, P, NKV * D).transpose(1, 0, 2))
    wo_pk = np.ascontiguousarray(
        wo.reshape(KT_HID, P, HID).transpose(1, 0, 2))
    return {"wq_pk": wq_pk, "wk_g": wk_g, "wv_pk": wv_pk, "wo_pk": wo_pk}


_COMPILED = None


def _get_compiled():
    global _COMPILED
    if _COMPILED is None:
        nc = bacc.Bacc("TRN2", target_bir_lowering=False, debug=False)
        _build(nc)
        nc.compile()
        _COMPILED = nc
    return _COMPILED


def kernel(hidden_states, Wq, Wk, Wv, Wo, _trace=False, _trace_kwargs=None):
    hs = np.asarray(hidden_states, dtype=np.float32).astype(BF)
    weights = _prep_weights(Wq, Wk, Wv, Wo)
    consts = _host_consts()
    nc = _get_compiled()
    in_maps = []
    for c in range(NCORES):
        # ship X pre-transposed AND pre-tiled ([p, k, tok]) so the kernel's
        # loads are plane-sliced contiguous DMAs
        shard = hs[BL * c: BL * (c + 1)].reshape(TOK, HID).T  # [HID, TOK]
        shard_pk = np.ascontiguousarray(
            shard.reshape(KT_HID, P, TOK).transpose(1, 0, 2))
        in_maps.append({"hidden_pk": shard_pk, **weights, **consts})
    res = run_bass_kernel_spmd(
        nc, in_maps, list(range(NCORES)), trace=_trace,
        **(_trace_kwargs or {}))
    outs = [np.asarray(r["out"]).astype(np.float32).reshape(BL, T, HID)
            for r in res.results]
    full = np.concatenate(outs, axis=0)
    if _trace:
        return full, res
    return full


# revision 10
# speedup vs baseline: 1.0808x; 1.0075x over previous
"""GQA attention kernel for Trainium2, data-parallel over batch on 8 NeuronCores.

Per-core problem (2 of 16 batches): X [1024tok, 1024] -> QKV proj -> RoPE ->
causal GQA attention (8 q heads, 4 kv heads, D=128) -> out proj [1024, 1024].

v3 layout strategy (PE-occupancy-driven; baseline v2 was ~146.4us):
  - All dram tensors are host-side pre-arranged to the exact sbuf fill
    layout ([p, k, n]) so every load DMA runs with 4-16KB contiguous
    descriptors, and loads are ordered by first use with the K-projection
    dependencies (X, then per-g Wk chunks) first across both HWDGE rings.
  - RoPE's partition-half swap is done ON THE DVE via stream_shuffle: the
    head dim of Wq/Wk (and cos/sin rows) is permuted so each rotate-half
    pair (i, i+64) lands 16 partitions apart inside one 32-partition
    quadrant (S = q.k is invariant under a consistent d-permutation).
    This removes all sbuf<->sbuf swap DMAs (3MB of ring traffic that used
    to compete with the weight loads) and makes rope a pure DVE chain.
  - The 1/sqrt(D) scale is folded into the exp's activation scale, so one
    UNSCALED cos/sin table pair [128, 512] is shared by Q and K rope and
    broadcast over the two batch chunks (0.25MB loaded vs 1MB).
  - Softmax denominator: the colsum matmul uses an ALL-ONES [128,128]
    stationary operand, so the psum result is the denominator already
    broadcast across partitions (same PE streaming cost, cheaper
    instruction shape than M=1) and normalization is a DVE
    reciprocal_approx_fast + one multiply -- no GPSIMD
    partition_broadcast; GPSIMD retires from the kernel entirely.
    (A single tensor_tensor divide would be cheaper still but the BIR
    verifier rejects divide on the DVE.)
  - PE warmup is dependency-light (ones@ones after a gpsimd memset) so it
    starts as soon as the PE queue comes up (~6.6us) and is sized to end
    exactly when X+Wk land (~15us), covering the whole load latency.
  - Output dram tensor is bf16 (host upcasts): halves store traffic, and
    the last two out-projection chunks drain in [128,128] quarters
    alternating ACT/DVE copies and sync/scalar rings to shorten the tail.
  - Engine assignment is FIFO-queue-driven (strict in-order queues):
      ACT:    exps, psum->sbuf copies (qraw/V/out tiles)
      DVE:    rope (shuffle+mul+mul+add), merged causal masks, divides
      GPSIMD: nothing (only the startup ones memset)
      sync/scalar DMA rings: loads first-use-ordered, then output stores
  - Same software pipeline as v2: S/exp emission runs a full head ahead of
    the colsum/PV consumers (deque, drain threshold 11), attention
    consumption lags Q-chain emission by 3 heads, rope DVE chains are
    emitted one head-slot late.
"""

import numpy as np
import ml_dtypes
from collections import deque
from contextlib import ExitStack

import concourse.bass as bass
import concourse.tile as tile
from concourse import bacc, mybir
from concourse.bass_utils import run_bass_kernel_spmd

B, T, HID = 16, 512, 1024
NH, NKV, D = 8, 4, 128
THETA = 10000.0
NCORES = 8
BL = B // NCORES          # local batches per core
TOK = BL * T              # local tokens
P = 128
KT_HID = HID // P         # 8 contraction tiles over hidden
NTQ = T // P              # 4 tk/tq tiles per sequence
NTOK_T = TOK // P         # 8 token tiles per core
GROUPS = NH // NKV        # 2 q heads per kv head
SCALE = 1.0 / float(np.sqrt(D))
FP32 = mybir.dt.float32
BF16 = mybir.dt.bfloat16
BF = ml_dtypes.bfloat16

# rope-pair permutation: old pair (i, i+64) -> within-quadrant pair
# (32q+j, 32q+16+j) with q=i//16, j=i%16, so one stream_shuffle mask
# (swap 16-partition halves of each 32-partition quadrant) does the
# rotate-half partition move on the DVE.
_DPERM = np.empty(D, dtype=np.int64)          # old index of each new slot
for _q in range(4):
    for _j in range(16):
        _DPERM[32 * _q + _j] = 16 * _q + _j
        _DPERM[32 * _q + 16 + _j] = 64 + 16 * _q + _j
SHUF_MASK = list(range(16, 32)) + list(range(0, 16))

WARM = 26                 # PE warmup matmuls (ones[P,P] @ ones[P,P])


def _host_consts():
    inv_freq = 1.0 / (THETA ** (np.arange(0, D, 2, dtype=np.float64) / D))
    freqs = np.outer(np.arange(T, dtype=np.float64), inv_freq)    # [T, 64]
    emb = np.concatenate([freqs, freqs], axis=-1)                 # [T, 128]
    cos = np.cos(emb).T                                           # [128, T]
    sin = np.sin(emb).T
    # rotate_half sign folded into sin: out = x*cos + shuffle(x)*sin_signed
    sin_signed = np.concatenate([-sin[:D // 2], sin[D // 2:]], axis=0)
    # transposed-S diagonal-block multiplicative mask: rows tk, cols tq;
    # valid iff tq >= tk
    mask_t = np.triu(np.ones((P, P), np.float32)).astype(BF)
    return {
        "cos_t": np.ascontiguousarray(cos[_DPERM]).astype(BF),
        "sin_t": np.ascontiguousarray(sin_signed[_DPERM]).astype(BF),
        "mask_t": mask_t,
    }


def _build(nc):
    hid = nc.dram_tensor("hidden_pk", [P, KT_HID, TOK], BF16,
                         kind="ExternalInput").ap()
    wq = nc.dram_tensor("wq_pk", [P, KT_HID, NH * D], BF16,
                        kind="ExternalInput").ap()
    wk = nc.dram_tensor("wk_g", [NKV, P, KT_HID, D], BF16,
                        kind="ExternalInput").ap()
    wv = nc.dram_tensor("wv_pk", [P, KT_HID, NKV * D], BF16,
                        kind="ExternalInput").ap()
    wo = nc.dram_tensor("wo_pk", [P, KT_HID, HID], BF16,
                        kind="ExternalInput").ap()
    cos_t = nc.dram_tensor("cos_t", [P, T], BF16, kind="ExternalInput").ap()
    sin_t = nc.dram_tensor("sin_t", [P, T], BF16, kind="ExternalInput").ap()
    mask_t = nc.dram_tensor("mask_t", [P, P], BF16, kind="ExternalInput").ap()
    out = nc.dram_tensor("out", [TOK, HID], BF16, kind="ExternalOutput").ap()

    with tile.TileContext(nc) as tc, ExitStack() as ctx:
        # ---- pools with cross-phase lifetimes ----
        consts = ctx.enter_context(tc.tile_pool(name="consts", bufs=1))

        cos_sb = consts.tile([P, T], BF16, tag="cos")
        sin_sb = consts.tile([P, T], BF16, tag="sin")
        maskt_sb = consts.tile([P, P], BF16, tag="maskt")
        ones_bf = consts.tile([P, P], BF16, tag="ones")
        # gpsimd comes up first (~6.1us) -> warmup deps ready earliest
        nc.gpsimd.memset(ones_bf, 1.0)

        qkvpool = ctx.enter_context(tc.tile_pool(name="qkv", bufs=1))
        qt_sb = qkvpool.tile([P, NH, BL, T], BF16, tag="qt")    # [d,h,b,t]
        kt_sb = qkvpool.tile([P, NKV, BL, T], BF16, tag="kt")   # [d,g,b,t]
        v_sb = qkvpool.tile([P, NTOK_T, NKV * D], BF16, tag="v")
        otpool = ctx.enter_context(tc.tile_pool(name="otpool", bufs=1))
        ot_sb = otpool.tile([P, NH, BL, T], BF16, tag="ot")     # [d,h,b,t]

        wpool = ctx.enter_context(tc.tile_pool(name="wpool", bufs=1))
        wq_sb = wpool.tile([P, KT_HID, NH * D], BF16, tag="wq")
        wk_sb = wpool.tile([P, NKV, KT_HID, D], BF16, tag="wk")
        wv_sb = wpool.tile([P, KT_HID, NKV * D], BF16, tag="wv")
        wo_sb = wpool.tile([P, KT_HID, HID], BF16, tag="wo")
        xt_sb = wpool.tile([P, KT_HID, TOK], BF16, tag="xt")    # [hid,k,tok]

        # ---- input loads: first-use order. Wk first (every K-chain matmul
        # needs its g-slice), then X plane-by-plane so the interleaved
        # K-chain wave can chase the DMA wavefront; everything contiguous
        # in dram per partition ----
        nc.sync.dma_start(out=wk_sb[:, 0], in_=wk[0])
        nc.scalar.dma_start(out=wk_sb[:, 1], in_=wk[1])
        nc.sync.dma_start(out=wk_sb[:, 2], in_=wk[2])
        nc.scalar.dma_start(out=wk_sb[:, 3], in_=wk[3])
        for k in range(KT_HID):
            eng = nc.sync if k % 2 == 0 else nc.scalar
            eng.dma_start(out=xt_sb[:, k, :], in_=hid[:, k, :])
        nc.sync.dma_start(out=cos_sb, in_=cos_t)
        nc.sync.dma_start(out=sin_sb, in_=sin_t)
        nc.scalar.dma_start(out=wv_sb, in_=wv)
        nc.sync.dma_start(out=wq_sb[:, 0:4, :], in_=wq[:, 0:4, :])
        nc.scalar.dma_start(out=wq_sb[:, 4:8, :], in_=wq[:, 4:8, :])
        nc.sync.dma_start(out=maskt_sb, in_=mask_t)
        nc.sync.dma_start(out=wo_sb[:, 0:4, :], in_=wo[:, 0:4, :])
        nc.scalar.dma_start(out=wo_sb[:, 4:8, :], in_=wo[:, 4:8, :])

        cos_bc = cos_sb[:, None, :].to_broadcast([P, BL, T])
        sin_bc = sin_sb[:, None, :].to_broadcast([P, BL, T])

        def _rope_dve(raw, out_sl, tmp_pool):
            """Full rope on the DVE: partition-half swap via stream_shuffle
            (head-dim permuted so pairs sit within 32-partition quadrants),
            then out = raw*cos + shuffled*sin_signed."""
            swp = tmp_pool.tile([P, BL, T], BF16, tag="rope_swp", bufs=2,
                                name="swp")
            nc.vector.stream_shuffle(swp, raw, SHUF_MASK)
            tmp = tmp_pool.tile([P, BL, T], BF16, tag="rope_tmp", bufs=2,
                                name="tmp")
            nc.vector.tensor_mul(tmp, swp, sin_bc)
            nc.vector.tensor_mul(out_sl, raw, cos_bc)
            nc.vector.tensor_add(out_sl, out_sl, tmp)

        # ---- phase A: warmup + K proj + V proj ----
        with ExitStack() as phase1:
            ropet = phase1.enter_context(tc.tile_pool(name="ropetA", bufs=2))
            psA = phase1.enter_context(
                tc.tile_pool(name="psA", bufs=8, space=bass.MemorySpace.PSUM))

            # PE warmup: dependency-free matmuls so the PE queue ramps the
            # HAM clock gate (1.2 -> 2.4 GHz) while Wk + the first X planes
            # are in flight; sized to end right as they land (~11.5us)
            wps = psA.tile([P, T], FP32, tag="projps", name="wps")
            for w in range(WARM):
                nc.tensor.matmul(wps[:, 0:P], ones_bf, ones_bf,
                                 start=True, stop=True, skip_group_check=True)

            # KT = Wk.T @ XT: ALL 8 (g, c) chains interleaved plane-by-plane
            # so the PE chases the X DMA wavefront (plane k is consumed by 8
            # matmuls ~1.7us while the next plane needs only ~0.7us to load)
            kraws = [ropet.tile([P, BL, T], BF16, tag="rope_raw", bufs=4,
                                name="kraw") for _ in range(NKV)]
            kps = {(g, c): psA.tile([P, T], FP32, tag="projps", name="kps")
                   for g in range(NKV) for c in range(BL)}
            for k in range(KT_HID):
                for (g, c), ps in kps.items():
                    nc.tensor.matmul(
                        ps,
                        wk_sb[:, g, k, :],
                        xt_sb[:, k, c * T:(c + 1) * T],
                        start=(k == 0), stop=(k == KT_HID - 1),
                        skip_group_check=True)
            for g in range(NKV):
                for c in range(BL):
                    nc.scalar.copy(kraws[g][:, c, :], kps[(g, c)])
                _rope_dve(kraws[g], kt_sb[:, g], ropet)
            # V natural: [tok, dkv]
            for tt in range(NTOK_T):
                ps = psA.tile([P, T], FP32, tag="projps")
                for k in range(KT_HID):
                    nc.tensor.matmul(
                        ps[:, :NKV * D],
                        xt_sb[:, k, tt * P:(tt + 1) * P],
                        wv_sb[:, k, :],
                        start=(k == 0), stop=(k == KT_HID - 1))
                nc.scalar.copy(v_sb[:, tt, :], ps[:, :NKV * D])

        # ---- phase B: Q proj interleaved with attention ----
        with ExitStack() as phase2:
            ropet = phase2.enter_context(tc.tile_pool(name="ropetB", bufs=2))
            ptpool = phase2.enter_context(tc.tile_pool(name="ptpool", bufs=5))
            stats = phase2.enter_context(tc.tile_pool(name="stats", bufs=3))
            psM = phase2.enter_context(
                tc.tile_pool(name="psM", bufs=1, space=bass.MemorySpace.PSUM))

            pend = deque()
            head_state = {}
            qraw_state = {}
            rope_pending = deque()

            def emit_qchain(h, c):
                ps = psM.tile([P, T], FP32, tag="qps", bufs=2, name="qps")
                for k in range(KT_HID):
                    nc.tensor.matmul(
                        ps,
                        wq_sb[:, k, h * P:(h + 1) * P],
                        xt_sb[:, k, c * T:(c + 1) * T],
                        start=(k == 0), stop=(k == KT_HID - 1))
                if c == 0:
                    qraw_state[h] = ropet.tile([P, BL, T], BF16,
                                               tag="rope_raw", bufs=2,
                                               name="qraw")
                qraw = qraw_state[h]
                nc.scalar.copy(qraw[:, c, :], ps)
                if c == BL - 1:
                    rope_pending.append((qraw, h))
                    del qraw_state[h]

            def emit_rope_dve():
                qraw, h = rope_pending.popleft()
                _rope_dve(qraw, qt_sb[:, h], ropet)

            def emit_item_S(b, h):
                """S matmuls + exps for all 4 tk-blocks of one (batch, head),
                then ONE merged causal-mask multiply: each PT_j plane stores
                columns [lo:T] shifted to plane-local [0:T-lo], so all four
                diagonal blocks line up at plane-local [0:P] and mask in a
                single strided DVE op."""
                g = h // GROUPS
                pt = ptpool.tile([P, NTQ, T], BF16, tag="pt", bufs=5,
                                 name="pt")
                for j in range(NTQ):
                    lo = j * P
                    st_ps = psM.tile([P, T], FP32, tag="sps", bufs=3,
                                     name="sps")
                    nc.tensor.matmul(
                        st_ps[:, lo:T],
                        kt_sb[:, g, b, lo:lo + P],
                        qt_sb[:, h, b, lo:T],
                        start=True, stop=True)
                    # exp -> PT_j, already transposed for the PV matmul;
                    # 1/sqrt(D) folded into the activation scale
                    # (no row-max: logits are O(1) by construction)
                    nc.scalar.activation(
                        out=pt[:, j, 0:T - lo], in_=st_ps[:, lo:T],
                        func=mybir.ActivationFunctionType.Exp,
                        bias=0.0, scale=SCALE)
                nc.vector.tensor_mul(
                    pt[:, :, 0:P], pt[:, :, 0:P],
                    maskt_sb[:, None, :].to_broadcast([P, NTQ, P]))
                for j in range(NTQ):
                    pend.append((b, h, j, pt))

            def drain_one():
                b, h, j, pt = pend.popleft()
                g = h // GROUPS
                lo = j * P
                st = head_state.get((b, h))
                if st is None:
                    o_ps_new = psM.tile([P, T], FP32, tag="ops", bufs=2,
                                        name="ops")
                    cs_ps_new = psM.tile([P, T], FP32, tag="cps", bufs=1,
                                         name="cps")
                    st = head_state[(b, h)] = (o_ps_new, cs_ps_new)
                o_ps, cs_ps = st
                # colsum with an ALL-ONES stationary: psum rows all carry
                # the denominator (partition-broadcast for free);
                # OT += V_j.T @ PT_j chained in the other psum
                nc.tensor.matmul(
                    cs_ps[:, lo:T] if j else cs_ps[:, :],
                    ones_bf,
                    pt[:, j, 0:T - lo],
                    start=(j == 0), stop=(j == NTQ - 1),
                    skip_group_check=True)
                nc.tensor.matmul(
                    o_ps[:, lo:T] if j else o_ps[:, :],
                    v_sb[:, b * NTQ + j, g * D:(g + 1) * D],
                    pt[:, j, 0:T - lo],
                    start=(j == 0), stop=(j == NTQ - 1),
                    skip_group_check=True)
                if j == NTQ - 1:
                    # per-head softmax normalization, fully on the DVE: the
                    # all-ones colsum already broadcast the denominator
                    # across partitions, so reciprocal + one multiply
                    rb = stats.tile([P, T], FP32, tag="rb", bufs=2,
                                    name="rb")
                    nc.vector.reciprocal_approx_fast(rb, cs_ps)
                    nc.vector.tensor_mul(ot_sb[:, h, b], o_ps, rb)
                    del head_state[(b, h)]

            def emit_att(b, h):
                emit_item_S(b, h)
                while len(pend) > 11:
                    drain_one()

            def emit_oproj(tt, cchunk):
                # one out-projection chunk: out[tt-block, chunk] as its own
                # k-chain, sharing the qps psum rotation with the (finished)
                # Q chains so it can interleave with the attention tail
                ps = psM.tile([P, T], FP32, tag="qps", bufs=2, name="opps")
                for k in range(KT_HID):
                    nc.tensor.matmul(
                        ps,
                        ot_sb[:, k, tt // NTQ,
                              (tt % NTQ) * P:(tt % NTQ + 1) * P],
                        wo_sb[:, k, cchunk * T:(cchunk + 1) * T],
                        start=(k == 0), stop=(k == KT_HID - 1))
                o_tile = stats.tile([P, T], BF16, tag="oout", bufs=3,
                                    name="o_tile")
                if tt >= NTOK_T - 1:
                    # final tiles: drain in quarters alternating both copy
                    # engines, stores issued from the idle sync/gpsimd
                    # queues (a dma_start parked in the ACT/DVE queue would
                    # block the next copy until the transfer finishes)
                    QT = T // 4
                    for q in range(4):
                        sl = slice(q * QT, (q + 1) * QT)
                        if q % 2 == 0:
                            nc.scalar.copy(o_tile[:, sl], ps[:, sl])
                        else:
                            nc.vector.tensor_copy(o_tile[:, sl], ps[:, sl])
                        eng = nc.sync if q % 2 == 0 else nc.gpsimd
                        base = cchunk * T
                        eng.dma_start(
                            out=out[tt * P:(tt + 1) * P,
                                    base + q * QT:base + (q + 1) * QT],
                            in_=o_tile[:, sl])
                    return
                # copies always on ACT: a PE-dependent copy on the DVE would
                # head-of-line delay the recips/norms that gate later chains;
                # stores always on the otherwise-idle sync queue
                nc.scalar.copy(o_tile, ps)
                nc.sync.dma_start(
                    out=out[tt * P:(tt + 1) * P,
                            cchunk * T:(cchunk + 1) * T],
                    in_=o_tile)

            # attention lags the Q chains by 3 heads and the DVE part of
            # each rope lags its chain by 1 slot: every engine-queue entry
            # has its dependencies resolved before it reaches the queue
            # head, so the strict-FIFO DVE queue never head-of-line blocks
            # the causal masks that gate the PE's colsum/PV matmuls
            LAG = 3
            for h in range(NH):
                if h >= LAG:
                    emit_att(0, h - LAG)
                emit_qchain(h, 0)
                if h >= LAG:
                    emit_att(1, h - LAG)
                if rope_pending and h >= 1:
                    emit_rope_dve()
                emit_qchain(h, 1)
            while rope_pending:
                emit_rope_dve()
            # tail: batch-0 attention first, then interleave out-projection
            # chunks (batch-0 token tiles first) with the remaining
            # ACT/DVE-bound attention so the PE stays dense
            for h in range(NH - LAG, NH):
                emit_att(0, h)
            emit_att(1, NH - LAG)
            while len(pend) > 4:
                drain_one()              # flush: norms (0, *) all emitted
            emit_oproj(0, 0)
            emit_oproj(0, 1)
            emit_att(1, NH - 2)
            emit_oproj(1, 0)
            emit_oproj(1, 1)
            emit_att(1, NH - 1)
            emit_oproj(2, 0)
            emit_oproj(2, 1)
            while len(pend) > 4:
                drain_one()              # flush: norm (1, NH-2) emitted
            emit_oproj(3, 0)
            while pend:
                drain_one()              # norm (1, NH-1)
            emit_oproj(3, 1)
            for tt in range(4, NTOK_T):
                emit_oproj(tt, 0)
                emit_oproj(tt, 1)


def _prep_weights(Wq, Wk, Wv, Wo):
    """Host-side: bf16, rope-pair permutation of the q/k head dims, and
    rearrangement to the sbuf fill layout [p, k, n] (contiguous per
    partition) so load DMA descriptors are large."""
    wq = np.asarray(Wq, dtype=np.float32).astype(BF)
    wk = np.asarray(Wk, dtype=np.float32).astype(BF)
    wv = np.asarray(Wv, dtype=np.float32).astype(BF)
    wo = np.asarray(Wo, dtype=np.float32).astype(BF)
    # permute the per-head d axis of Wq/Wk
    wq = wq.reshape(HID, NH, D)[:, :, _DPERM].reshape(HID, NH * D)
    wk = wk.reshape(HID, NKV, D)[:, :, _DPERM].reshape(HID, NKV * D)
    wq_pk = np.ascontiguousarray(
        wq.reshape(KT_HID, P, NH * D).transpose(1, 0, 2))
    wk_g = np.ascontiguousarray(
        wk.reshape(KT_HID, P, NKV, D).transpose(2, 1, 0, 3))
    wv_pk = np.ascontiguousarray(
        wv.reshape(KT_HID, P, NKV * D).transpose(1, 0, 2))
    wo_pk = np.ascontiguousarray(
        wo.reshape(KT_HID, P, HID).transpose(1, 0, 2))
    return {"wq_pk": wq_pk, "wk_g": wk_g, "wv_pk": wv_pk, "wo_pk": wo_pk}


_COMPILED = None


def _get_compiled():
    global _COMPILED
    if _COMPILED is None:
        nc = bacc.Bacc("TRN2", target_bir_lowering=False, debug=False)
        _build(nc)
        nc.compile()
        _COMPILED = nc
    return _COMPILED


def kernel(hidden_states, Wq, Wk, Wv, Wo, _trace=False, _trace_kwargs=None):
    hs = np.asarray(hidden_states, dtype=np.float32).astype(BF)
    weights = _prep_weights(Wq, Wk, Wv, Wo)
    consts = _host_consts()
    nc = _get_compiled()
    in_maps = []
    for c in range(NCORES):
        # ship X pre-transposed AND pre-tiled ([p, k, tok]) so the kernel's
        # loads are plane-sliced contiguous DMAs
        shard = hs[BL * c: BL * (c + 1)].reshape(TOK, HID).T  # [HID, TOK]
        shard_pk = np.ascontiguousarray(
            shard.reshape(KT_HID, P, TOK).transpose(1, 0, 2))
        in_maps.append({"hidden_pk": shard_pk, **weights, **consts})
    res = run_bass_kernel_spmd(
        nc, in_maps, list(range(NCORES)), trace=_trace,
        **(_trace_kwargs or {}))
    outs = [np.asarray(r["out"]).astype(np.float32).reshape(BL, T, HID)
            for r in res.results]
    full = np.concatenate(outs, axis=0)
    if _trace:
        return full, res
    return full


# revision 14
# speedup vs baseline: 1.1235x; 1.0395x over previous
"""GQA attention kernel for Trainium2, data-parallel over batch on 8 NeuronCores.

Per-core problem (2 of 16 batches): X [1024tok, 1024] -> QKV proj -> RoPE ->
causal GQA attention (8 q heads, 4 kv heads, D=128) -> out proj [1024, 1024].

v3 layout strategy (PE-occupancy-driven; baseline v2 was ~146.4us):
  - All dram tensors are host-side pre-arranged to the exact sbuf fill
    layout ([p, k, n]) so every load DMA runs with 4-16KB contiguous
    descriptors, and loads are ordered by first use with the K-projection
    dependencies (X, then per-g Wk chunks) first across both HWDGE rings.
  - RoPE's partition-half swap is done ON THE DVE via stream_shuffle: the
    head dim of Wq/Wk (and cos/sin rows) is permuted so each rotate-half
    pair (i, i+64) lands 16 partitions apart inside one 32-partition
    quadrant (S = q.k is invariant under a consistent d-permutation).
    This removes all sbuf<->sbuf swap DMAs (3MB of ring traffic that used
    to compete with the weight loads) and makes rope a pure DVE chain.
  - The 1/sqrt(D) scale is folded into the exp's activation scale, so one
    UNSCALED cos/sin table pair [128, 512] is shared by Q and K rope and
    broadcast over the two batch chunks (0.25MB loaded vs 1MB).
  - Softmax denominator: the colsum matmul uses an ALL-ONES [128,128]
    stationary operand, so the psum result is the denominator already
    broadcast across partitions (same PE streaming cost, cheaper
    instruction shape than M=1) and normalization is a DVE
    reciprocal_approx_fast + one multiply -- no GPSIMD
    partition_broadcast; GPSIMD retires from the kernel entirely.
    (A single tensor_tensor divide would be cheaper still but the BIR
    verifier rejects divide on the DVE.)
  - PE warmup is dependency-light (ones@ones after a gpsimd memset) so it
    starts as soon as the PE queue comes up (~6.6us) and is sized to end
    exactly when X+Wk land (~15us), covering the whole load latency.
  - Output dram tensor is bf16 (host upcasts): halves store traffic, and
    the last two out-projection chunks drain in [128,128] quarters
    alternating ACT/DVE copies and sync/scalar rings to shorten the tail.
  - Engine assignment is FIFO-queue-driven (strict in-order queues):
      ACT:    exps, psum->sbuf copies (qraw/V/out tiles)
      DVE:    rope (shuffle+mul+mul+add), merged causal masks, divides
      GPSIMD: nothing (only the startup ones memset)
      sync/scalar DMA rings: loads first-use-ordered, then output stores
  - Same software pipeline as v2: S/exp emission runs a full head ahead of
    the colsum/PV consumers (deque, drain threshold 11), attention
    consumption lags Q-chain emission by 3 heads, rope DVE chains are
    emitted one head-slot late.
"""

import numpy as np
import ml_dtypes
from collections import deque
from contextlib import ExitStack

import concourse.bass as bass
import concourse.tile as tile
from concourse import bacc, mybir
from concourse.bass_utils import run_bass_kernel_spmd

B, T, HID = 16, 512, 1024
NH, NKV, D = 8, 4, 128
THETA = 10000.0
NCORES = 8
BL = B // NCORES          # local batches per core
TOK = BL * T              # local tokens
P = 128
KT_HID = HID // P         # 8 contraction tiles over hidden
NTQ = T // P              # 4 tk/tq tiles per sequence
NTOK_T = TOK // P         # 8 token tiles per core
GROUPS = NH // NKV        # 2 q heads per kv head
SCALE = 1.0 / float(np.sqrt(D))
FP32 = mybir.dt.float32
BF16 = mybir.dt.bfloat16
BF = ml_dtypes.bfloat16

# rope-pair permutation: old pair (i, i+64) -> within-quadrant pair
# (32q+j, 32q+16+j) with q=i//16, j=i%16, so one stream_shuffle mask
# (swap 16-partition halves of each 32-partition quadrant) does the
# rotate-half partition move on the DVE.
_DPERM = np.empty(D, dtype=np.int64)          # old index of each new slot
for _q in range(4):
    for _j in range(16):
        _DPERM[32 * _q + _j] = 16 * _q + _j
        _DPERM[32 * _q + 16 + _j] = 64 + 16 * _q + _j
SHUF_MASK = list(range(16, 32)) + list(range(0, 16))

WARM = 56                 # PE warmup matmuls (ones[P,P] @ ones[P,P])


def _host_consts():
    inv_freq = 1.0 / (THETA ** (np.arange(0, D, 2, dtype=np.float64) / D))
    freqs = np.outer(np.arange(T, dtype=np.float64), inv_freq)    # [T, 64]
    emb = np.concatenate([freqs, freqs], axis=-1)                 # [T, 128]
    cos = np.cos(emb).T                                           # [128, T]
    sin = np.sin(emb).T
    # rotate_half sign folded into sin: out = x*cos + shuffle(x)*sin_signed
    sin_signed = np.concatenate([-sin[:D // 2], sin[D // 2:]], axis=0)
    # transposed-S diagonal-block multiplicative mask: rows tk, cols tq;
    # valid iff tq >= tk
    mask_t = np.triu(np.ones((P, P), np.float32)).astype(BF)
    return {
        "cos_t": np.ascontiguousarray(cos[_DPERM]).astype(BF),
        "sin_t": np.ascontiguousarray(sin_signed[_DPERM]).astype(BF),
        "mask_t": mask_t,
    }


def _build(nc):
    hid = nc.dram_tensor("hidden_pk", [P, KT_HID, TOK], BF16,
                         kind="ExternalInput").ap()
    wq = nc.dram_tensor("wq_pk", [P, KT_HID, NH * D], BF16,
                        kind="ExternalInput").ap()
    wk = nc.dram_tensor("wk_g", [NKV, P, KT_HID, D], BF16,
                        kind="ExternalInput").ap()
    wv = nc.dram_tensor("wv_pk", [P, KT_HID, NKV * D], BF16,
                        kind="ExternalInput").ap()
    wo = nc.dram_tensor("wo_pk", [P, KT_HID, HID], BF16,
                        kind="ExternalInput").ap()
    cos_t = nc.dram_tensor("cos_t", [P, T], BF16, kind="ExternalInput").ap()
    sin_t = nc.dram_tensor("sin_t", [P, T], BF16, kind="ExternalInput").ap()
    mask_t = nc.dram_tensor("mask_t", [P, P], BF16, kind="ExternalInput").ap()
    out = nc.dram_tensor("out", [TOK, HID], BF16, kind="ExternalOutput").ap()

    with tile.TileContext(nc) as tc, ExitStack() as ctx:
        # ---- pools with cross-phase lifetimes ----
        consts = ctx.enter_context(tc.tile_pool(name="consts", bufs=1))

        cos_sb = consts.tile([P, T], BF16, tag="cos")
        sin_sb = consts.tile([P, T], BF16, tag="sin")
        maskt_sb = consts.tile([P, P], BF16, tag="maskt")
        ones_bf = consts.tile([P, P], BF16, tag="ones")
        # gpsimd comes up first (~6.1us) -> warmup deps ready earliest
        nc.gpsimd.memset(ones_bf, 1.0)

        qkvpool = ctx.enter_context(tc.tile_pool(name="qkv", bufs=1))
        qt_sb = qkvpool.tile([P, NH, BL, T], BF16, tag="qt")    # [d,h,b,t]
        kt_sb = qkvpool.tile([P, NKV, BL, T], BF16, tag="kt")   # [d,g,b,t]
        v_sb = qkvpool.tile([P, NTOK_T, NKV * D], BF16, tag="v")
        otpool = ctx.enter_context(tc.tile_pool(name="otpool", bufs=1))
        ot_sb = otpool.tile([P, NH, BL, T], BF16, tag="ot")     # [d,h,b,t]

        wpool = ctx.enter_context(tc.tile_pool(name="wpool", bufs=1))
        wq_sb = wpool.tile([P, KT_HID, NH * D], BF16, tag="wq")
        wk_sb = wpool.tile([P, NKV, KT_HID, D], BF16, tag="wk")
        wv_sb = wpool.tile([P, KT_HID, NKV * D], BF16, tag="wv")
        wo_sb = wpool.tile([P, KT_HID, HID], BF16, tag="wo")
        xt_sb = wpool.tile([P, KT_HID, TOK], BF16, tag="xt")    # [hid,k,tok]

        # ---- input loads: first-use order, aware that the DMA engines
        # alternate between the two rings' FIFOs (global completion order
        # ~= zip of the two lists) and that a completed transfer is only
        # CONSUMABLE ~2.5us later (completion->semaphore->engine latency).
        # Wave 1 of the K proj needs wk0+wk1+xt0 first; wk2/wk3 are only
        # needed when wave 2 starts (~7us later) ----
        nc.sync.dma_start(out=wk_sb[:, 0], in_=wk[0])
        nc.scalar.dma_start(out=wk_sb[:, 1], in_=wk[1])
        for k in range(KT_HID):
            eng = nc.sync if k % 2 == 0 else nc.scalar
            eng.dma_start(out=xt_sb[:, k, :], in_=hid[:, k, :])
        nc.sync.dma_start(out=wk_sb[:, 2], in_=wk[2])
        nc.scalar.dma_start(out=wk_sb[:, 3], in_=wk[3])
        nc.sync.dma_start(out=cos_sb, in_=cos_t)
        nc.scalar.dma_start(out=sin_sb, in_=sin_t)
        nc.scalar.dma_start(out=wv_sb, in_=wv)
        nc.sync.dma_start(out=wq_sb[:, 0:4, :], in_=wq[:, 0:4, :])
        nc.scalar.dma_start(out=wq_sb[:, 4:8, :], in_=wq[:, 4:8, :])
        nc.sync.dma_start(out=maskt_sb, in_=mask_t)
        nc.sync.dma_start(out=wo_sb[:, 0:4, :], in_=wo[:, 0:4, :])
        nc.scalar.dma_start(out=wo_sb[:, 4:8, :], in_=wo[:, 4:8, :])

        cos_bc = cos_sb[:, None, :].to_broadcast([P, BL, T])
        sin_bc = sin_sb[:, None, :].to_broadcast([P, BL, T])

        def _rope_dve(raw, out_sl, tmp_pool):
            """Full rope on the DVE: partition-half swap via stream_shuffle
            (head-dim permuted so pairs sit within 32-partition quadrants),
            then out = raw*cos + shuffled*sin_signed."""
            swp = tmp_pool.tile([P, BL, T], BF16, tag="rope_swp", bufs=2,
                                name="swp")
            nc.vector.stream_shuffle(swp, raw, SHUF_MASK)
            tmp = tmp_pool.tile([P, BL, T], BF16, tag="rope_tmp", bufs=2,
                                name="tmp")
            nc.vector.tensor_mul(tmp, swp, sin_bc)
            nc.vector.tensor_mul(out_sl, raw, cos_bc)
            nc.vector.tensor_add(out_sl, out_sl, tmp)

        # ---- phase A: warmup + K proj + V proj ----
        with ExitStack() as phase1:
            ropet = phase1.enter_context(tc.tile_pool(name="ropetA", bufs=2))
            psA = phase1.enter_context(
                tc.tile_pool(name="psA", bufs=8, space=bass.MemorySpace.PSUM))

            # PE warmup: dependency-free matmuls so the PE queue ramps the
            # HAM clock gate (1.2 -> 2.4 GHz) while Wk + the first X planes
            # are in flight; sized to end right as they land (~11.5us)
            wps = psA.tile([P, T], FP32, tag="projps", name="wps")
            for w in range(WARM):
                nc.tensor.matmul(wps[:, 0:P], ones_bf, ones_bf,
                                 start=True, stop=True, skip_group_check=True)

            # KT = Wk.T @ XT: the 8 (g, c) chains run as two 4-chain waves
            # interleaved plane-by-plane so the PE chases the X DMA
            # wavefront (plane k feeds 4 matmuls ~0.85us while the next
            # plane needs ~0.7us to load); wave 1 only needs wk0/wk1 so it
            # can start as soon as xt0 is consumable
            kraws = [ropet.tile([P, BL, T], BF16, tag="rope_raw", bufs=4,
                                name="kraw") for _ in range(NKV)]
            for wave in range(2):
                gs = (0, 1) if wave == 0 else (2, 3)
                kps = {(g, c): psA.tile([P, T], FP32, tag="projps",
                                        name="kps")
                       for g in gs for c in range(BL)}
                for k in range(KT_HID):
                    for (g, c), ps in kps.items():
                        nc.tensor.matmul(
                            ps,
                            wk_sb[:, g, k, :],
                            xt_sb[:, k, c * T:(c + 1) * T],
                            start=(k == 0), stop=(k == KT_HID - 1),
                            skip_group_check=True)
                for g in gs:
                    for c in range(BL):
                        nc.scalar.copy(kraws[g][:, c, :], kps[(g, c)])
                    _rope_dve(kraws[g], kt_sb[:, g], ropet)
            # V natural: [tok, dkv]
            for tt in range(NTOK_T):
                ps = psA.tile([P, T], FP32, tag="projps")
                for k in range(KT_HID):
                    nc.tensor.matmul(
                        ps[:, :NKV * D],
                        xt_sb[:, k, tt * P:(tt + 1) * P],
                        wv_sb[:, k, :],
                        start=(k == 0), stop=(k == KT_HID - 1))
                nc.scalar.copy(v_sb[:, tt, :], ps[:, :NKV * D])

        # ---- phase B: Q proj interleaved with attention ----
        with ExitStack() as phase2:
            ropet = phase2.enter_context(tc.tile_pool(name="ropetB", bufs=2))
            ptpool = phase2.enter_context(tc.tile_pool(name="ptpool", bufs=5))
            stats = phase2.enter_context(tc.tile_pool(name="stats", bufs=3))
            psM = phase2.enter_context(
                tc.tile_pool(name="psM", bufs=1, space=bass.MemorySpace.PSUM))

            pend = deque()
            head_state = {}
            qraw_state = {}
            rope_pending = deque()

            def emit_qchain(h, c):
                ps = psM.tile([P, T], FP32, tag="qps", bufs=2, name="qps")
                for k in range(KT_HID):
                    nc.tensor.matmul(
                        ps,
                        wq_sb[:, k, h * P:(h + 1) * P],
                        xt_sb[:, k, c * T:(c + 1) * T],
                        start=(k == 0), stop=(k == KT_HID - 1))
                if c == 0:
                    qraw_state[h] = ropet.tile([P, BL, T], BF16,
                                               tag="rope_raw", bufs=2,
                                               name="qraw")
                qraw = qraw_state[h]
                nc.scalar.copy(qraw[:, c, :], ps)
                if c == BL - 1:
                    rope_pending.append((qraw, h))
                    del qraw_state[h]

            def emit_rope_dve():
                qraw, h = rope_pending.popleft()
                _rope_dve(qraw, qt_sb[:, h], ropet)

            def emit_item_S(b, h):
                """S matmuls + exps for all 4 tk-blocks of one (batch, head),
                then ONE merged causal-mask multiply: each PT_j plane stores
                columns [lo:T] shifted to plane-local [0:T-lo], so all four
                diagonal blocks line up at plane-local [0:P] and mask in a
                single strided DVE op."""
                g = h // GROUPS
                pt = ptpool.tile([P, NTQ, T], BF16, tag="pt", bufs=5,
                                 name="pt")
                for j in range(NTQ):
                    lo = j * P
                    st_ps = psM.tile([P, T], FP32, tag="sps", bufs=3,
                                     name="sps")
                    nc.tensor.matmul(
                        st_ps[:, lo:T],
                        kt_sb[:, g, b, lo:lo + P],
                        qt_sb[:, h, b, lo:T],
                        start=True, stop=True)
                    # exp -> PT_j, already transposed for the PV matmul;
                    # 1/sqrt(D) folded into the activation scale
                    # (no row-max: logits are O(1) by construction)
                    nc.scalar.activation(
                        out=pt[:, j, 0:T - lo], in_=st_ps[:, lo:T],
                        func=mybir.ActivationFunctionType.Exp,
                        bias=0.0, scale=SCALE)
                nc.vector.tensor_mul(
                    pt[:, :, 0:P], pt[:, :, 0:P],
                    maskt_sb[:, None, :].to_broadcast([P, NTQ, P]))
                for j in range(NTQ):
                    pend.append((b, h, j, pt))

            def drain_one():
                b, h, j, pt = pend.popleft()
                g = h // GROUPS
                lo = j * P
                st = head_state.get((b, h))
                if st is None:
                    o_ps_new = psM.tile([P, T], FP32, tag="ops", bufs=2,
                                        name="ops")
                    cs_ps_new = psM.tile([P, T], FP32, tag="cps", bufs=1,
                                         name="cps")
                    st = head_state[(b, h)] = (o_ps_new, cs_ps_new)
                o_ps, cs_ps = st
                # colsum with an ALL-ONES stationary: psum rows all carry
                # the denominator (partition-broadcast for free);
                # OT += V_j.T @ PT_j chained in the other psum
                nc.tensor.matmul(
                    cs_ps[:, lo:T] if j else cs_ps[:, :],
                    ones_bf,
                    pt[:, j, 0:T - lo],
                    start=(j == 0), stop=(j == NTQ - 1),
                    skip_group_check=True)
                nc.tensor.matmul(
                    o_ps[:, lo:T] if j else o_ps[:, :],
                    v_sb[:, b * NTQ + j, g * D:(g + 1) * D],
                    pt[:, j, 0:T - lo],
                    start=(j == 0), stop=(j == NTQ - 1),
                    skip_group_check=True)
                if j == NTQ - 1:
                    # per-head softmax normalization, fully on the DVE: the
                    # all-ones colsum already broadcast the denominator
                    # across partitions, so reciprocal + one multiply
                    rb = stats.tile([P, T], FP32, tag="rb", bufs=2,
                                    name="rb")
                    nc.vector.reciprocal_approx_fast(rb, cs_ps)
                    nc.vector.tensor_mul(ot_sb[:, h, b], o_ps, rb)
                    del head_state[(b, h)]

            def emit_att(b, h):
                emit_item_S(b, h)
                while len(pend) > 11:
                    drain_one()

            def emit_oproj(tt, cchunk):
                # one out-projection chunk: out[tt-block, chunk] as its own
                # k-chain, sharing the qps psum rotation with the (finished)
                # Q chains so it can interleave with the attention tail
                ps = psM.tile([P, T], FP32, tag="qps", bufs=2, name="opps")
                for k in range(KT_HID):
                    nc.tensor.matmul(
                        ps,
                        ot_sb[:, k, tt // NTQ,
                              (tt % NTQ) * P:(tt % NTQ + 1) * P],
                        wo_sb[:, k, cchunk * T:(cchunk + 1) * T],
                        start=(k == 0), stop=(k == KT_HID - 1))
                o_tile = stats.tile([P, T], BF16, tag="oout", bufs=3,
                                    name="o_tile")
                if tt == NTOK_T - 1 and cchunk == 1:
                    # very last chunk: drain in quarters, ALL copies emitted
                    # before ANY store so no dma_start (which occupies the
                    # issuing engine for ~1us) ever delays a copy; stores
                    # split across the two HWDGE rings (gpsimd/vector
                    # dma_start would be a slow software DIRECT2D copy)
                    QT = T // 4
                    base = cchunk * T
                    for q in range(4):
                        sl = slice(q * QT, (q + 1) * QT)
                        if q % 2 == 0:
                            nc.scalar.copy(o_tile[:, sl], ps[:, sl])
                        else:
                            nc.vector.tensor_copy(o_tile[:, sl], ps[:, sl])
                    for q in range(4):
                        sl = slice(q * QT, (q + 1) * QT)
                        eng = nc.sync if q % 2 == 0 else nc.scalar
                        eng.dma_start(
                            out=out[tt * P:(tt + 1) * P,
                                    base + q * QT:base + (q + 1) * QT],
                            in_=o_tile[:, sl])
                    return
                # copies always on ACT: a PE-dependent copy on the DVE would
                # head-of-line delay the recips/norms that gate later chains;
                # stores always on the otherwise-idle sync queue
                nc.scalar.copy(o_tile, ps)
                nc.sync.dma_start(
                    out=out[tt * P:(tt + 1) * P,
                            cchunk * T:(cchunk + 1) * T],
                    in_=o_tile)

            # attention lags the Q chains by 3 heads and the DVE part of
            # each rope lags its chain by 1 slot: every engine-queue entry
            # has its dependencies resolved before it reaches the queue
            # head, so the strict-FIFO DVE queue never head-of-line blocks
            # the causal masks that gate the PE's colsum/PV matmuls
            LAG = 3
            for h in range(NH):
                if h >= LAG:
                    emit_att(0, h - LAG)
                emit_qchain(h, 0)
                if h >= LAG:
                    emit_att(1, h - LAG)
                if rope_pending and h >= 1:
                    emit_rope_dve()
                emit_qchain(h, 1)
            while rope_pending:
                emit_rope_dve()
            # tail: batch-0 attention first, then interleave out-projection
            # chunks (batch-0 token tiles first) with the remaining
            # ACT/DVE-bound attention so the PE stays dense
            for h in range(NH - LAG, NH):
                emit_att(0, h)
            emit_att(1, NH - LAG)
            while len(pend) > 4:
                drain_one()              # flush: norms (0, *) all emitted
            emit_oproj(0, 0)
            emit_oproj(0, 1)
            emit_att(1, NH - 2)
            emit_oproj(1, 0)
            emit_oproj(1, 1)
            emit_att(1, NH - 1)
            emit_oproj(2, 0)
            emit_oproj(2, 1)
            while len(pend) > 4:
                drain_one()              # flush: norm (1, NH-2) emitted
            emit_oproj(3, 0)
            while pend:
                drain_one()              # norm (1, NH-1)
            emit_oproj(3, 1)
            for tt in range(4, NTOK_T):
                emit_oproj(tt, 0)
                emit_oproj(tt, 1)


def _prep_weights(Wq, Wk, Wv, Wo):
    """Host-side: bf16, rope-pair permutation of the q/k head dims, and
    rearrangement to the sbuf fill layout [p, k, n] (contiguous per
    partition) so load DMA descriptors are large."""
    wq = np.asarray(Wq, dtype=np.float32).astype(BF)
    wk = np.asarray(Wk, dtype=np.float32).astype(BF)
    wv = np.asarray(Wv, dtype=np.float32).astype(BF)
    wo = np.asarray(Wo, dtype=np.float32).astype(BF)
    # permute the per-head d axis of Wq/Wk
    wq = wq.reshape(HID, NH, D)[:, :, _DPERM].reshape(HID, NH * D)
    wk = wk.reshape(HID, NKV, D)[:, :, _DPERM].reshape(HID, NKV * D)
    wq_pk = np.ascontiguousarray(
        wq.reshape(KT_HID, P, NH * D).transpose(1, 0, 2))
    wk_g = np.ascontiguousarray(
        wk.reshape(KT_HID, P, NKV, D).transpose(2, 1, 0, 3))
    wv_pk = np.ascontiguousarray(
        wv.reshape(KT_HID, P, NKV * D).transpose(1, 0, 2))
    wo_pk = np.ascontiguousarray(
        wo.reshape(KT_HID, P, HID).transpose(1, 0, 2))
    return {"wq_pk": wq_pk, "wk_g": wk_g, "wv_pk": wv_pk, "wo_pk": wo_pk}


_COMPILED = None


def _get_compiled():
    global _COMPILED
    if _COMPILED is None:
        nc = bacc.Bacc("TRN2", target_bir_lowering=False, debug=False)
        _build(nc)
        nc.compile()
        _COMPILED = nc
    return _COMPILED


def kernel(hidden_states, Wq, Wk, Wv, Wo, _trace=False, _trace_kwargs=None):
    hs = np.asarray(hidden_states, dtype=np.float32).astype(BF)
    weights = _prep_weights(Wq, Wk, Wv, Wo)
    consts = _host_consts()
    nc = _get_compiled()
    in_maps = []
    for c in range(NCORES):
        # ship X pre-transposed AND pre-tiled ([p, k, tok]) so the kernel's
        # loads are plane-sliced contiguous DMAs
        shard = hs[BL * c: BL * (c + 1)].reshape(TOK, HID).T  # [HID, TOK]
        shard_pk = np.ascontiguousarray(
            shard.reshape(KT_HID, P, TOK).transpose(1, 0, 2))
        in_maps.append({"hidden_pk": shard_pk, **weights, **consts})
    res = run_bass_kernel_spmd(
        nc, in_maps, list(range(NCORES)), trace=_trace,
        **(_trace_kwargs or {}))
    outs = [np.asarray(r["out"]).astype(np.float32).reshape(BL, T, HID)
            for r in res.results]
    full = np.concatenate(outs, axis=0)
    if _trace:
        return full, res
    return full
